# revision 19
# baseline (speedup 1.0000x reference)
"""Trainium2 Bass kernel for a 2-layer GCN (CascadePredictionModel).

Model (per reference):
    src/dst = edge_index + self loops; deg over dst; norm_e = rsqrt(deg[src])*rsqrt(deg[dst])
    gcn(h, W, b) = segment_sum(norm * (h@W)[src], dst) + b
    h1 = relu(gcn(x,  W1, b1))
    h2 = relu(gcn(h1, W2, b2))
    pred = noise @ W3 + b3
    out = concat([h2, pred])            # [N+M, C]

Distribution strategy (8 NeuronCores, SPMD single NEFF):
  - Destination nodes are 1D-partitioned: core k owns dst rows [1250k, 1250k+1250).
  - Feature matmul Z = h@W computed per-core for owned rows (weights replicated,
    fp16), cast to fp8-e4m3.  (fp8 on the aggregation path measures 4.2e-3 rel
    err vs the fp32 reference on the real inputs — 5x inside the 2e-2 gate; the
    averaging over ~17 in-edges cancels most of the quantization noise.)
  - The AllGather is SLICED in two (local rows 0-639 = dst tiles 0-4, rows
    640-1249 = tiles 5-9) and pipelined: AG of slice 0 runs while the feature
    matmul computes tiles 5-9, and the slice-0 row gathers + aggregation
    matmuls run while AG of slice 1 is still in flight.  (Measured: each
    exposed AllGather dependency costs ~50-70us on HW; sliced+overlapped it
    mostly disappears.)
  - Aggregation per dst tile of 128: dma_gather pulls the (per-tile deduped,
    src-sorted, per-slice) source rows into SBUF as [128, nchunk, 512] fp8; PE
    accumulates psum += S_c^T @ G_c over chunk PAIRS in fp8 DoubleRow mode
    (2 contraction chunks per instruction at 2x rate), where S is a host-built
    fp8 selection matrix holding the edge norms (sums parallel edges).  Bias is
    added with one identity-matmul against a broadcast-bias tile; relu on
    ScalarE.  Gathers are merged per (slice, tile-pair) — the ~1us fixed SWDGE
    prep cost per dma_gather instruction on the Pool engine made many small
    gathers a bottleneck.
  - h1^T for the layer-2 matmul is built per-tile with PE transposes right
    after the layer-1 relu, so layer-2 feature matmuls pipeline behind the
    layer-1 aggregation.
  - pred rows are sharded 250/core and run inside the AllGather bubbles.

The whole per-invocation graph structure (edge sort, per-tile/per-slice dedup,
S matrices, gather indices) is built on the host; per-tile chunk counts are
maxed over cores so the single SPMD NEFF is identical on all 8 cores.
"""

import math
import time
from contextlib import ExitStack

import ml_dtypes
import numpy as np

F8 = ml_dtypes.float8_e4m3  # TRN fp8e4 (IEEE-ish, max +-240)

N, E, C, MPRED = 10000, 160000, 512, 2000
P = 8                 # cores
NPC = N // P          # 1250 nodes per core
TPB = 128             # dst-tile width
NT = (NPC + TPB - 1) // TPB   # 10 tiles / core (last has 98 dsts)
NPAD = NT * TPB       # 1280
PRED_PC = MPRED // P  # 250 pred rows per core
KT = C // 128         # 4 contraction tiles

SL = 2                                   # source slices (pipelined AllGather)
SBASE = (0, 640)                         # local row base of each slice
SROWS = (640, NPC - 640)                 # 640 rows (tiles 0-4), 610 (tiles 5-9)
GROUPS = ((0, 1), (2, 3), (4, 5), (6, 7), (8, 9))  # tiles per gather

_prog_cache: dict[tuple, tuple] = {}
LAST_RESULTS = None  # BassKernelResults of the most recent run (for test.py)


def _slice_layout(NCHs):
    """Global chunk numbering: slice-major, gather-group-major, tile-major.

    Returns (base, gspec, NCTOT):
      base[(s, t)] : first global chunk index of tile t's slice-s chunks
      gspec        : [(s, gi, c0, nch, ((t, toff, cnt), ...)), ...] one entry
                     per dma_gather (slice s, tile group gi, global chunk
                     range [c0, c0+nch), per-tile local offsets)
    """
    base = {}
    gspec = []
    off = 0
    for s in range(SL):
        for gi, g in enumerate(GROUPS):
            c0 = off
            tl = []
            for t in g:
                base[(s, t)] = off
                tl.append((t, off - c0, NCHs[s][t]))
                off += NCHs[s][t]
            gspec.append((s, gi, c0, off - c0, tuple(tl)))
    return base, gspec, off


# ---------------------------------------------------------------- host tables
def _host_tables(edge_index):
    """Build per-core gather indices + selection matrices.

    Returns (NCHs, idxs_list, S_list):
      NCHs         : NCHs[s][t] = chunk count of (slice s, dst tile t), maxed
                     over cores so the SPMD program is core-independent
      idxs_list[k] : [128, NIDX//16] int16  (16-partition wrap, tiled x8)
      S_list[k]    : [128, NCTOT, 128] fp8,
                     S[p, base[(s,t)]+c, m] = sum of norms of edges
                     (src=u_{s,t}[c*128+p] -> dst=k*NPC+t*128+m)
    """
    ei = np.asarray(edge_index).astype(np.int64)
    src = np.concatenate([ei[0], np.arange(N, dtype=np.int64)])
    dst = np.concatenate([ei[1], np.arange(N, dtype=np.int64)])
    deg = np.bincount(dst, minlength=N).astype(np.float64)
    dis = np.where(deg > 0, 1.0 / np.sqrt(np.maximum(deg, 1.0)), 0.0)
    norm = (dis[src] * dis[dst]).astype(np.float32)

    order = np.lexsort((src, dst))
    src_s, dst_s, norm_s = src[order], dst[order], norm[order]

    nchs = [[1] * NT for _ in range(SL)]
    per_tile = []   # [(t, (u_s0, u_s1), es, dloc, en)] x (P*NT)
    for k in range(P):
        for t in range(NT):
            lo = k * NPC + t * TPB
            hi = min((k + 1) * NPC, lo + TPB)
            m0 = np.searchsorted(dst_s, lo)
            m1 = np.searchsorted(dst_s, hi)
            es = src_s[m0:m1]
            u = np.unique(es)
            r = u % NPC
            us = tuple(u[(r >= SBASE[s]) & (r < SBASE[s] + SROWS[s])]
                       for s in range(SL))
            for s in range(SL):
                nchs[s][t] = max(nchs[s][t], (len(us[s]) + 127) // 128)
            per_tile.append((t, us, es, dst_s[m0:m1] - lo, norm_s[m0:m1]))
    NCHs = tuple(tuple(x) for x in nchs)
    base, _, NCTOT = _slice_layout(NCHs)
    NIDX = NCTOT * 128

    idxs_list, S_list = [], []
    for k in range(P):
        idxs = np.zeros(NIDX, dtype=np.int64)
        S = np.zeros((NIDX, TPB), dtype=np.float32)
        for (t, us, es, dloc, en) in per_tile[k * NT:(k + 1) * NT]:
            eo = es // NPC
            er = es - eo * NPC
            pos = np.empty(len(es), dtype=np.int64)
            for s in range(SL):
                b = base[(s, t)] * 128
                u = us[s]
                uo = u // NPC
                ur = u - uo * NPC
                idxs[b:b + len(u)] = uo * SROWS[s] + (ur - SBASE[s])
                m = (er >= SBASE[s]) & (er < SBASE[s] + SROWS[s])
                pos[m] = b + np.searchsorted(u, es[m])
            np.add.at(S, (pos, dloc), en)
        wrapped = np.tile(idxs.reshape(-1, 16).T, (8, 1)).astype(np.int16)
        S_host = np.ascontiguousarray(
            S.reshape(NCTOT, 128, TPB).transpose(1, 0, 2)
        ).astype(F8)
        idxs_list.append(np.ascontiguousarray(wrapped))
        S_list.append(S_host)
    return NCHs, idxs_list, S_list


# ---------------------------------------------------------------- device prog
def _build_program(NCHs, sim1core=False, loops=1, no_cc=False, no_gather=False,
                   decouple_cc=False, nqueues=4, scratch=49152):
    """sim1core=True builds a single-core timing variant for TimelineSim:
    collectives are replaced by a DRAM->DRAM DMA of the own-shard slice
    (the dependency carrier), everything else identical.
    loops>1 repeats the whole compute body (timing calibration: the wall-time
    slope over `loops` isolates the per-iteration device span from the
    per-execute dispatch overhead)."""
    import concourse.bacc as bacc
    import concourse.mybir as mybir
    import concourse.tile as tile

    f16, f32, i16 = mybir.dt.float16, mybir.dt.float32, mybir.dt.int16
    f8 = mybir.dt.float8e4
    DR = mybir.MatmulPerfMode.DoubleRow
    Relu = mybir.ActivationFunctionType.Relu
    Copy = mybir.ActivationFunctionType.Copy
    base, gspec, NCTOT = _slice_layout(NCHs)
    NIDX = NCTOT * 128

    nc = bacc.Bacc(
        "TRN2", target_bir_lowering=False, debug=False,
        num_devices=(1 if sim1core else P),
        num_swdge_queues=nqueues,
        dynamic_dma_scratch_size=scratch,
    )

    xT_d = nc.dram_tensor("xT", [128, KT, NPAD], f16, kind="ExternalInput")
    w1_d = nc.dram_tensor("W1t", [128, KT, C], f16, kind="ExternalInput")
    w2_d = nc.dram_tensor("W2t", [128, KT, C], f16, kind="ExternalInput")
    w3_d = nc.dram_tensor("W3t", [128, KT, C], f16, kind="ExternalInput")
    s_d = nc.dram_tensor("S", [128, NCTOT, 128], f8, kind="ExternalInput")
    idx_d = nc.dram_tensor("idxs", [128, NIDX // 16], i16, kind="ExternalInput")
    bias_d = nc.dram_tensor("biasbc", [128, 3, C], f16, kind="ExternalInput")
    bias8_d = nc.dram_tensor("biasbc8", [128, 2, C], f8, kind="ExternalInput")
    ident_d = nc.dram_tensor("ident", [128, 128], f16, kind="ExternalInput")
    ident8_d = nc.dram_tensor("ident8", [128, 128], f8, kind="ExternalInput")
    nzT_d = nc.dram_tensor("noiseT", [128, KT, 256], f16, kind="ExternalInput")
    out_d = nc.dram_tensor("out", [NPC + PRED_PC, C], f16, kind="ExternalOutput")

    zb = [[nc.dram_tensor(f"zb{l}_{s}", [SROWS[s], C], f8, kind="Internal")
           for s in range(SL)] for l in range(2)]
    zf = [[nc.dram_tensor(f"zf{l}_{s}", [P * SROWS[s], C], f8, kind="Internal",
                          addr_space=("Local" if sim1core else "Shared"))
           for s in range(SL)] for l in range(2)]
    zffake = (
        [[nc.dram_tensor(f"zff{l}_{s}", [P * SROWS[s], C], f8, kind="Internal")
          for s in range(SL)] for l in range(2)]
        if decouple_cc else None
    )

    with tile.TileContext(nc) as tc, ExitStack() as ctx:
        consts = ctx.enter_context(tc.tile_pool(name="consts", bufs=1))
        zpool = ctx.enter_context(tc.tile_pool(name="z", bufs=6))
        gpool = ctx.enter_context(tc.tile_pool(name="g", bufs=6))
        hpool = ctx.enter_context(tc.tile_pool(name="h", bufs=3))
        opool = ctx.enter_context(tc.tile_pool(name="o", bufs=3))
        fpsum = ctx.enter_context(tc.tile_pool(name="fps", bufs=2, space="PSUM"))
        apsum = ctx.enter_context(tc.tile_pool(name="aps", bufs=3, space="PSUM"))
        tpsum = ctx.enter_context(tc.tile_pool(name="tps", bufs=2, space="PSUM"))

        xT = consts.tile([128, KT, NPAD], f16, tag="xT")
        W1 = consts.tile([128, KT, C], f16, tag="W1")
        W2 = consts.tile([128, KT, C], f16, tag="W2")
        W3 = consts.tile([128, KT, C], f16, tag="W3")
        St = consts.tile([128, NCTOT, 128], f8, tag="S")
        idxt = consts.tile([128, NIDX // 16], i16, tag="idx")
        biast = consts.tile([128, 3, C], f16, tag="bias")
        biast8 = consts.tile([128, 2, C], f8, tag="bias8")
        ident = consts.tile([128, 128], f16, tag="ident")
        ident8 = consts.tile([128, 128], f8, tag="ident8")
        nzT = consts.tile([128, KT, 256], f16, tag="nzT")
        h1T = consts.tile([128, KT, NPAD], f16, tag="h1T")

        # layer-1 feature operands first — S/idxs aren't needed until after
        # AllGather-0, so their big loads must not delay the first matmuls.
        nc.sync.dma_start(xT[:], xT_d[:])
        nc.sync.dma_start(W1[:], w1_d[:])
        nc.sync.dma_start(W3[:], w3_d[:])
        nc.sync.dma_start(biast[:], bias_d[:])
        nc.sync.dma_start(biast8[:], bias8_d[:])
        nc.sync.dma_start(ident[:], ident_d[:])
        nc.sync.dma_start(ident8[:], ident8_d[:])
        nc.sync.dma_start(nzT[:], nzT_d[:])
        nc.sync.dma_start(idxt[:], idx_d[:])
        nc.sync.dma_start(St[:], s_d[:])
        nc.sync.dma_start(W2[:], w2_d[:])

        def feature_tile(lhsT, Wt, l, t):
            ps = fpsum.tile([128, C], f32, tag="fps")
            for g in range(KT):
                nc.tensor.matmul(
                    ps[:],
                    lhsT[:, g, t * 128:(t + 1) * 128],
                    Wt[:, g, :],
                    start=(g == 0),
                    stop=(g == KT - 1),
                )
            zt = zpool.tile([128, C], f8, tag="z")
            nc.scalar.activation(zt[:], ps[:], Copy)
            s = 0 if (t * 128) < SBASE[1] else 1
            lo = t * 128 - SBASE[s]
            w = min(128, SROWS[s] - lo)
            nc.sync.dma_start(zb[l][s][lo:lo + w, :], zt[:w, :])

        rg = [list(range(P))]

        def allgather(l, s):
            if no_cc or sim1core:
                # AllGather stand-in: just the dependency-carrying own-shard
                # copy (also the TimelineSim timing model; real AG wall is
                # measured separately via calibrate.py).
                nc.sync.dma_start(zf[l][s][:SROWS[s], :], zb[l][s][:])
            else:
                nc.gpsimd.collective_compute(
                    "AllGather",
                    bacc.mybir.AluOpType.bypass,
                    replica_groups=rg,
                    ins=[zb[l][s][:]],
                    outs=[zf[l][s][:]],
                )

        _qn = [0]

        def gather_one(l, s, gi, c0, nch, Gt):
            qn = _qn[0] % nqueues
            _qn[0] += 1
            src_d = zffake[l][s] if decouple_cc else zf[l][s]
            G = gpool.tile([128, nch, C], f8, tag="g")
            if no_gather:
                nc.vector.memset(G[:, 0, 0:16], 0.0)
            else:
                nc.gpsimd.dma_gather(
                    G[:],
                    src_d[:],
                    idxt[:, c0 * 8: (c0 + nch) * 8],
                    nch * 128,
                    nch * 128,
                    C,
                    single_packet=(nch * 128 <= 1024),
                    queue_num=qn,
                )
            Gt[(s, gi)] = G

        def agg_chunks(ps, cglob, G, toff, n, first):
            """Accumulate chunks [cglob, cglob+n) (G local cols [toff,
            toff+n)) into ps; fp8 DoubleRow over pairs."""
            i = 0
            while i < n:
                if i + 1 < n:
                    nc.tensor.matmul(
                        ps[:],
                        St[:, cglob + i: cglob + i + 2, :],
                        G[:, toff + i: toff + i + 2, :],
                        start=first, stop=False, perf_mode=DR,
                    )
                    i += 2
                else:
                    nc.tensor.matmul(
                        ps[:], St[:, cglob + i, :], G[:, toff + i, :],
                        start=first, stop=False,
                    )
                    i += 1
                first = False
            return first

        GIDX = {t: next(i for i, g in enumerate(GROUPS) if t in g)
                for t in range(NT)}
        TOFF = {(e[0], t): to for e in gspec for (t, to, cnt) in e[4]}

        def agg_tile(lidx, t, Gt, emit_out):
            ps = apsum.tile([128, C], f32, tag="aps")
            first = True
            for s in range(SL):
                first = agg_chunks(ps, base[(s, t)], Gt[(s, GIDX[t])],
                                   TOFF[(s, t)], NCHs[s][t], first)
            nc.tensor.matmul(
                ps[:], ident8[:], biast8[:, lidx, :], start=False, stop=True
            )
            emit_out(t, ps)

        # pred = noise @ W3 + b3 (no relu), 250 rows/core — one tile emitted in
        # each AllGather-0 bubble so the PE has work while waiting.
        def pred_tile(mt):
            ps = fpsum.tile([128, C], f32, tag="fps")
            for g in range(KT):
                nc.tensor.matmul(
                    ps[:],
                    nzT[:, g, mt * 128:(mt + 1) * 128],
                    W3[:, g, :],
                    start=(g == 0),
                    stop=False,
                )
            nc.tensor.matmul(ps[:], ident[:], biast[:, 2, :], start=False, stop=True)
            ot = opool.tile([128, C], f16, tag="o")
            nc.scalar.activation(ot[:], ps[:], Copy)
            w = min(128, PRED_PC - mt * 128)
            nc.sync.dma_start(
                out_d[NPC + mt * 128: NPC + mt * 128 + w, :], ot[:w, :]
            )

        def l1_out(t, ps):
            # relu -> fp16, then PE-transpose the [128, 512] tile into h1T so
            # the layer-2 feature matmul for this node tile can start at once.
            ht = hpool.tile([128, C], f16, tag="h")
            nc.scalar.activation(ht[:], ps[:], Relu)
            for g in range(KT):
                pt = tpsum.tile([128, 128], f16, tag="tps")
                nc.tensor.transpose(pt[:], ht[:, g * 128:(g + 1) * 128], ident[:])
                nc.vector.tensor_copy(h1T[:, g, t * 128:(t + 1) * 128], pt[:])

        def l2_out(t, ps):
            ot = opool.tile([128, C], f16, tag="o")
            nc.scalar.activation(ot[:], ps[:], Relu)
            w = NPC - t * 128 if t == NT - 1 else 128
            nc.sync.dma_start(out_d[t * 128: t * 128 + w, :], ot[:w, :])

        def layer(lidx, lhsT, Wt, emit_out):
            for t in range(5):
                feature_tile(lhsT, Wt, lidx, t)
            allgather(lidx, 0)
            pred_tile(lidx)
            for t in range(5, NT):
                feature_tile(lhsT, Wt, lidx, t)
            Gt = {}
            # first slice-0 gather, then AG-1: the collective chain isn't
            # stuck behind all the slice-0 SWDGE preps on the Pool queue, yet
            # the first gather still fires the moment AG-0 lands.
            (s0, gi0, c00, nch0, _) = gspec[0]
            gather_one(lidx, s0, gi0, c00, nch0, Gt)
            allgather(lidx, 1)
            for (s, gi, c0, nch, _) in gspec[1:]:
                gather_one(lidx, s, gi, c0, nch, Gt)
            for t in range(NT):
                agg_tile(lidx, t, Gt, emit_out)

        for _rep in range(loops):
            layer(0, xT, W1, l1_out)
            layer(1, h1T, W2, l2_out)

    nc.compile()
    return nc


def _get_program(NCHs):
    if NCHs not in _prog_cache:
        _prog_cache[NCHs] = _build_program(NCHs)
    return _prog_cache[NCHs]


# ---------------------------------------------------------------- entry point
def _prepare(x, edge_index, W1, b1, W2, b2, W3, b3, noise, num_missing_nodes=None,
             **_ignored):
    """Host preprocessing: returns (nc, in_maps)."""
    x = np.asarray(x, dtype=np.float32)
    W1 = np.asarray(W1, dtype=np.float32)
    W2 = np.asarray(W2, dtype=np.float32)
    W3 = np.asarray(W3, dtype=np.float32)
    b1 = np.asarray(b1, dtype=np.float32)
    b2 = np.asarray(b2, dtype=np.float32)
    b3 = np.asarray(b3, dtype=np.float32)
    noise = np.asarray(noise, dtype=np.float32)

    NCHs, idxs_list, S_list = _host_tables(edge_index)
    nc = _get_program(NCHs)

    def wtiles(W):
        # [512, 512] -> [128, KT, 512] fp16
        return np.ascontiguousarray(
            W.reshape(KT, 128, C).transpose(1, 0, 2)
        ).astype(np.float16)

    biasbc = np.ascontiguousarray(
        np.broadcast_to(np.stack([b1, b2, b3])[None, :, :], (128, 3, C))
    ).astype(np.float16)
    biasbc8 = np.ascontiguousarray(
        np.broadcast_to(np.stack([b1, b2])[None, :, :], (128, 2, C))
    ).astype(F8)
    identity = np.eye(128, dtype=np.float16)
    identity8 = np.eye(128, dtype=F8)
    w1t, w2t, w3t = wtiles(W1), wtiles(W2), wtiles(W3)

    in_maps = []
    for k in range(P):
        xs = np.zeros((NPAD, C), dtype=np.float16)
        xs[:NPC] = x[k * NPC:(k + 1) * NPC].astype(np.float16)
        xT = np.ascontiguousarray(
            xs.T.reshape(KT, 128, NPAD).transpose(1, 0, 2)
        )
        nz = np.zeros((256, C), dtype=np.float16)
        nz[:PRED_PC] = noise[k * PRED_PC:(k + 1) * PRED_PC].astype(np.float16)
        nzT = np.ascontiguousarray(nz.T.reshape(KT, 128, 256).transpose(1, 0, 2))
        in_maps.append({
            "xT": xT,
            "W1t": w1t,
            "W2t": w2t,
            "W3t": w3t,
            "S": S_list[k],
            "idxs": idxs_list[k],
            "biasbc": biasbc,
            "biasbc8": biasbc8,
            "ident": identity,
            "ident8": identity8,
            "noiseT": nzT,
        })

    return nc, in_maps


def _assemble(results):
    out = np.empty((N + MPRED, C), dtype=np.float32)
    for k in range(P):
        o = results[k]["out"].astype(np.float32)
        out[k * NPC:(k + 1) * NPC] = o[:NPC]
        out[N + k * PRED_PC: N + (k + 1) * PRED_PC] = o[NPC:NPC + PRED_PC]
    return out


def kernel(x, edge_index, W1, b1, W2, b2, W3, b3, noise, num_missing_nodes=None,
           **_ignored):
    from concourse.bass_utils import run_bass_kernel_spmd

    nc, in_maps = _prepare(x, edge_index, W1, b1, W2, b2, W3, b3, noise,
                           num_missing_nodes)
    res = run_bass_kernel_spmd(nc, in_maps, core_ids=list(range(P)))
    global LAST_RESULTS
    LAST_RESULTS = res
    return _assemble(res.results)


if __name__ == "__main__":
    t0 = time.time()
    rng = np.random.default_rng(0)
    inputs = {
        "x": rng.standard_normal((N, C), dtype=np.float32),
        "edge_index": rng.integers(0, N, (2, E)).astype(np.int32),
        "W1": rng.standard_normal((C, C), dtype=np.float32) * 0.05,
        "b1": np.zeros(C, np.float32),
        "W2": rng.standard_normal((C, C), dtype=np.float32) * 0.05,
        "b2": np.zeros(C, np.float32),
        "W3": rng.standard_normal((C, C), dtype=np.float32) * 0.05,
        "b3": np.zeros(C, np.float32),
        "noise": rng.standard_normal((MPRED, C), dtype=np.float32),
        "num_missing_nodes": MPRED,
    }
    out = kernel(**inputs)
    print("kernel done", out.shape, time.time() - t0, "s")
